# revision 33
# baseline (speedup 1.0000x reference)
"""Trainium2 Bass kernel for an nn.DecoderBlock (pre-LN GPT block).

Reference computation (per batch element, fp32):
    h  = LN(x; g1,be1);  q,k,v = per-head projections of h
    y  = causal-softmax(q k^T / sqrt(hd)) v ;  x1 = x + y @ w_proj + b_proj
    h2 = LN(x1; g2,be2); out = x1 + gelu_tanh(h2 @ w_fc + b_fc) @ w_cp + b_cp

Shapes: B=8, T=1024, D=768, H=12, HD=64, F=3072.

Strategy: pure data parallelism — batch element b runs on core b (B == n_cores
== 8); the decoder block is independent per batch element so no collectives are
needed.  On-chip, all activations are kept *feature-major* ([D, T]: features on
partitions, tokens on the free axis) so chained matmuls need no transposes:
    out^T[n, t] = sum_d W[d, n] * A^T[d, t]   (lhsT = W as stored, rhs = A^T)
Attention scores are computed transposed (S^T[t, q]) so the softmax-weighted
probabilities land directly in the [t, q] layout the P@V matmul needs as its
moving operand; the two heads sharing a 128-partition group issue their K=64
score matmuls back-to-back so the PE runs them concurrently in disjoint
row-groups.  The softmax denominator comes from augmenting V with a
ones-column (row HD of the PV output is sum_t P[t,q]).  Softmax max-subtraction
is skipped: post-LN scores are O(5) so fp32 exp cannot overflow.

Perf notes (vs the first working version):
  * No GpSimd anywhere: partition broadcasts are K=1/K=2 PE matmuls against
    ones/pattern vectors; the causal-mask multiply runs on DVE.  GpSimd ops
    carried ~1us semaphore overhead each and serialized whole attention pairs.
  * PV matmuls run two iterations behind their score matmuls so the scalar
    engine's exp latency is hidden by score+filler matmuls.
  * LayerNorm stats stay in row layout ([1, T] psum rows from +-1/D ones
    matmuls); the per-token factors are broadcast across partitions by the PE.
  * MLP: the next fc tile's matmuls are emitted between a ph group and its
    dependent cp group, hiding the gelu; PSUM = 6 held cp accumulators + 2
    cycling scratch banks.
  * 1/sqrt(hd) folded into wq host-side; residual carried in bf16 (X1bf);
    ACT tables (Square/Sqrt/Exp/Gelu) prewarmed during the x DMA.
"""

import numpy as np
import ml_dtypes

import concourse.bass as bass
import concourse.mybir as mybir
import concourse.tile as tile
from concourse import bacc

BF16 = mybir.dt.bfloat16
F32 = mybir.dt.float32
AF = mybir.ActivationFunctionType
OP = mybir.AluOpType

# Full-problem dimensions (hardcoded; harness contract).
B, T, D, H = 8, 1024, 768, 12
HD = D // H
F = 4 * D
EPS = 1e-5
N_CORES = 8


# --------------------------------------------------------------------------
# Bass program builder
# --------------------------------------------------------------------------
def build_decoder_nc(T=T, D=D, H=H, F=F, TQ=512, with_bias=False, eps=EPS,
                     gelu_func=AF.Gelu_apprx_tanh, debug=False):
    """Build the single-core Bass program (same program runs SPMD on all cores).

    DRAM I/O layouts (all prepared host-side):
      xT    [D, T]             f32   x^T (feature-major)
      wq,wk [MC,128,KC,128]    bf16  packed lhsT tiles (LN1 affine + qk scale
                                     folded in)
      wv    [128,KC,D]         bf16  rhs layout for token-major V
      wp    [MC,128,KC,128]    bf16  w_proj packed
      wf    [FC,128,KC,128]    bf16  w_fc packed (LN2 affine folded in)
      wc    [FC,128,MC,128]    bf16  w_cp packed fc-major (plain reshape)
      *_b   [1, N]             bf16  folded bias rows (only if with_bias)
      outT  [D, T]             f32   output^T
    """
    assert D % 128 == 0 and F % 128 == 0 and T % TQ == 0 and TQ % 128 == 0
    TS = min(512, T)           # token chunk for projections/LN stats
    assert T % TS == 0 and TS == TQ
    KC = D // 128          # contraction chunks over D
    FC = F // 128          # chunks over MLP hidden
    MC = D // 128          # output-feature chunks over D
    NT = T // 128          # key/token chunks of 128
    NQ = T // TQ           # query chunks of TQ
    NTQ = TQ // 128
    HPC = 128 // HD        # heads per 128-partition group (2 for HD=64)
    VS = HD + 1            # V columns per head incl. ones-column
    assert H % HPC == 0 and HPC == 2 and NQ == 2
    NPAIR = H // HPC
    assert NPAIR == MC

    nc = bacc.Bacc()

    # ---- DRAM I/O ----
    xT = nc.dram_tensor("xT", [D, T], F32, kind="ExternalInput")
    wq_d = nc.dram_tensor("wq", [MC, 128, KC, 128], BF16, kind="ExternalInput")
    wk_d = nc.dram_tensor("wk", [MC, 128, KC, 128], BF16, kind="ExternalInput")
    wv_d = nc.dram_tensor("wv", [128, KC, D], BF16, kind="ExternalInput")
    wp_d = nc.dram_tensor("wp", [MC, 128, KC, 128], BF16, kind="ExternalInput")
    wf_d = nc.dram_tensor("wf", [FC, 128, KC, 128], BF16, kind="ExternalInput")
    wc_d = nc.dram_tensor("wc", [FC, 128, MC, 128], BF16, kind="ExternalInput")
    bias_d = {}
    if with_bias:
        for nm, width in (("bq", D), ("bk", D), ("bv", D), ("bp", D),
                          ("bf", F), ("bc", D)):
            bias_d[nm] = nc.dram_tensor(nm, [1, width], BF16,
                                        kind="ExternalInput")
    outT = nc.dram_tensor("outT", [D, T], F32, kind="ExternalOutput")
    outT_t = outT[:].rearrange("(o p) t -> p o t", p=128)
    dbg = {}
    if debug:
        NT_ = T // 128
        VS_ = HD + 1
        for nm, shp in (("dALN", [128, D // 128, T]), ("dQT", [128, D // 128, T]),
                        ("dKT", [128, D // 128, T]), ("dVt", [128, NT_, H * VS_]),
                        ("dYT", [128, D // 128, T]), ("dX1", [128, D // 128, T]),
                        ("dA2", [128, D // 128, T])):
            dbg[nm] = nc.dram_tensor(nm, shp, BF16, kind="ExternalOutput")
        dbg["dBS"] = nc.dram_tensor("dBS", [128, (H // 2) * 2, TQ], BF16,
                                    kind="ExternalOutput")

    # ---- constants (embedded in the NEFF) ----
    ones_bf = nc.inline_tensor(np.ones((1, T), ml_dtypes.bfloat16), "ones_bf")
    # stats ones-columns, mean factors folded: row0 = -1/D (mu), row1 = +1/D
    oc_np = np.empty((128, 2), ml_dtypes.bfloat16)
    oc_np[:, 0] = -1.0 / D
    oc_np[:, 1] = 1.0 / D
    onescol = nc.inline_tensor(oc_np, "onescol")
    # triangular mask for the diagonal 128x128 score blocks: 1 if i <= j
    m_np = (np.arange(128)[:, None] <= np.arange(128)[None, :]).astype(
        ml_dtypes.bfloat16)
    masks_d = nc.inline_tensor(m_np, "masks")

    with tile.TileContext(nc) as tc:
        with (
            tc.tile_pool(name="persist", bufs=1) as pp,
            tc.tile_pool(name="wts", bufs=3) as wpool,
            tc.tile_pool(name="work", bufs=3) as wkp,
            tc.tile_pool(name="small", bufs=2) as sp,
            tc.tile_pool(name="ps", bufs=1, space="PSUM") as ps,
        ):
            # ---- persistent SBUF tensors ----
            X = pp.tile([128, KC, T], F32, tag="X", name="X")
            Xbf = pp.tile([128, KC, T], BF16, tag="Xbf", name="Xbf")
            ALN = pp.tile([128, KC, T], BF16, tag="ALN", name="ALN")
            QT = pp.tile([128, KC, T], BF16, tag="QT", name="QT")
            KT = pp.tile([128, KC, T], BF16, tag="KT", name="KT")
            Vt = pp.tile([128, NT, H * VS], BF16, tag="Vt", name="Vt")
            YT = pp.tile([128, KC, T], BF16, tag="YT", name="YT")
            X1bf = pp.tile([128, KC, T], BF16, tag="X1bf", name="X1bf")
            A2 = pp.tile([128, KC, T], BF16, tag="A2", name="A2")

            onesb_s = None
            if with_bias:
                onesb_s = pp.tile([1, T], BF16, tag="onesb", name="onesb_s")
                nc.sync.dma_start(out=onesb_s, in_=ones_bf[:])
            onescol_s = pp.tile([128, 2], BF16, tag="onescol",
                                name="onescol_s")
            nc.sync.dma_start(out=onescol_s, in_=onescol[:])
            onesrow_s = pp.tile([1, 128], BF16, tag="onesrow",
                                name="onesrow_s")
            nc.sync.dma_start(out=onesrow_s, in_=ones_bf[0:1, 0:128])
            eps_p = pp.tile([1, 1], F32, tag="eps", name="eps_p")
            nc.vector.memset(eps_p, eps)
            masks_s = pp.tile([128, 128], BF16, tag="masks", name="masks_s")
            nc.sync.dma_start(out=masks_s, in_=masks_d[:])
            biases = {}
            for nm, dten in bias_d.items():
                bt = pp.tile(list(dten.shape), BF16, tag=nm, name=f"{nm}_s")
                nc.sync.dma_start(out=bt, in_=dten[:])
                biases[nm] = bt

            # ---- ACT table prewarm (overlaps the x DMA) ----
            warm = pp.tile([1, 5], F32, tag="warm", name="warm")
            nc.vector.memset(warm, 1.0)

            def prewarm(i, fn):
                nc.scalar.activation(out=warm[0:1, i:i + 1],
                                     in_=warm[0:1, i:i + 1], func=fn)

            prewarm(0, AF.Square)
            prewarm(1, AF.Sqrt)
            prewarm(2, AF.Copy)

            # V weights resident; Vt ones-columns
            wv_t = pp.tile([128, KC, D], BF16, tag="wv", name="wv_t")
            nc.sync.dma_start(out=wv_t, in_=wv_d[:])
            for h in range(H):
                nc.vector.memset(Vt[:, :, h * VS + HD: h * VS + HD + 1], 1.0)

            # ---- load x^T (per-kc pipelining: cast as each chunk lands) ----
            xT_t = xT[:].rearrange("(o p) t -> p o t", p=128)
            for kc in range(KC):
                nc.sync.dma_start(out=X[:, kc, :], in_=xT_t[:, kc, :])
                nc.vector.tensor_copy(out=Xbf[:, kc, :], in_=X[:, kc, :])

            # ---- LayerNorm (row layout; PE broadcasts; no GpSimd) ----
            # st[0,:] = -mu, st[32,:] = E[x^2] via +-1/D ones matmuls
            # (engine APs must start at partition 0/32/64).
            def ln_stats_kc(srcbf, tci, st, kc):
                tsl = slice(tci * TS, (tci + 1) * TS)
                sqc = wkp.tile([128, TS], BF16, tag="sqc", bufs=3,
                               name="sqc")
                nc.scalar.activation(out=sqc, in_=srcbf[:, kc, tsl],
                                     func=AF.Square)
                nc.tensor.matmul(
                    st[0:1, :], onescol_s[:, 0:1], srcbf[:, kc, tsl],
                    start=(kc == 0), stop=(kc == KC - 1))
                nc.tensor.matmul(
                    st[32:33, :], onescol_s[:, 1:2], sqc,
                    start=(kc == 0), stop=(kc == KC - 1))

            def ln_stats(srcbf, tci):
                st = ps.tile([33, TS], F32, tag="B", bufs=2, name="st")
                for kc in range(KC):
                    ln_stats_kc(srcbf, tci, st, kc)
                return st

            def ln_apply(st, srcbf, dst, tci):
                """Row chain: rstd/-mu*rstd rows -> PE broadcast -> DVE."""
                tsl = slice(tci * TS, (tci + 1) * TS)
                musq = sp.tile([1, TS], F32, tag="frow", bufs=3, name="musq")
                nc.scalar.activation(out=musq, in_=st[0:1, :],
                                     func=AF.Square)
                sdev = sp.tile([1, TS], F32, tag="frow", bufs=3, name="sdev")
                nc.vector.tensor_tensor(sdev, st[32:33, :], musq, OP.subtract)
                nc.scalar.activation(out=sdev, in_=sdev,
                                     func=AF.Sqrt, bias=eps_p[:])
                rstd = sp.tile([1, TS], F32, tag="frow", bufs=3, name="rstd")
                nc.vector.reciprocal_approx_fast(out=rstd, in_=sdev)
                rbf = sp.tile([1, TS], BF16, tag="brow", bufs=3, name="rbf")
                nc.vector.tensor_copy(out=rbf, in_=rstd)
                nbf = sp.tile([1, TS], BF16, tag="brow", bufs=3, name="nbf")
                # nbf = (-mu) * rstd
                nc.vector.tensor_tensor(nbf, st[0:1, :], rstd, OP.mult)
                bcR = ps.tile([128, TS], F32, tag="A", bufs=6, name="bcR")
                bcN = ps.tile([128, TS], F32, tag="A", bufs=6, name="bcN")
                nc.tensor.matmul(bcR, onesrow_s[0:1, :], rbf,
                                 start=True, stop=True)
                nc.tensor.matmul(bcN, onesrow_s[0:1, :], nbf,
                                 start=True, stop=True)
                # psum->sbuf on the scalar engine (closer to PSUM; keeps the
                # big DVE multiplies in packed bf16-SBUF mode)
                bcRs = wkp.tile([128, TS], BF16, tag="bcRs", bufs=2,
                                name="bcRs")
                bcNs = wkp.tile([128, TS], BF16, tag="bcNs", bufs=2,
                                name="bcNs")
                nc.scalar.copy(out=bcRs, in_=bcR)
                nc.scalar.copy(out=bcNs, in_=bcN)
                tmp = wkp.tile([128, KC, TS], BF16, tag="lntmp", bufs=2,
                               name="lntmp")
                nc.vector.tensor_tensor(
                    tmp, srcbf[:, :, tsl],
                    bcRs[:, None, :].to_broadcast((128, KC, TS)), OP.mult)
                nc.vector.tensor_tensor(
                    dst[:, :, tsl], tmp,
                    bcNs[:, None, :].to_broadcast((128, KC, TS)), OP.add)

            # LN1: both chunks' stats stream per-kc with the x DMA, then the
            # row chains; Exp/Gelu tables load during the stats matmuls.
            st0 = ps.tile([33, TS], F32, tag="B", bufs=2, name="st0")
            st1 = ps.tile([33, TS], F32, tag="B", bufs=2, name="st1")
            for kc in range(KC):
                ln_stats_kc(Xbf, 0, st0, kc)
                ln_stats_kc(Xbf, 1, st1, kc)
            ln_apply(st0, Xbf, ALN, 0)
            ln_apply(st1, Xbf, ALN, 1)
            prewarm(3, AF.Exp)
            prewarm(4, gelu_func)

            # ---- QKV / V / proj unit builders ----
            def bias_mm(psum, bias_t, msl, tsl):
                if bias_t is None:
                    return True
                nc.tensor.matmul(psum, bias_t[0:1, msl], onesb_s[0:1, tsl],
                                 start=True, stop=False)
                return False

            def make_filler(mc):
                """Filler units (closures) for pair mc's QKV + V matmuls."""
                msl = slice(mc * 128, (mc + 1) * 128)
                units = []
                for nm, wten, dstT in (("bq", wq_d, QT), ("bk", wk_d, KT)):
                    wt = wpool.tile([128, KC, 128], BF16, tag="w_qk", bufs=3,
                                    name="wt_qk")
                    nc.sync.dma_start(out=wt, in_=wten[mc])
                    for tci in range(NQ):
                        def qkv_unit(nm=nm, wt=wt, dstT=dstT, tci=tci):
                            tsl = slice(tci * TS, (tci + 1) * TS)
                            pq = ps.tile([128, TS], F32, tag="A", bufs=6,
                                         name="pq")
                            st = bias_mm(pq, biases.get(nm), msl, tsl)
                            for kc in range(KC):
                                nc.tensor.matmul(
                                    pq, wt[:, kc, :], ALN[:, kc, tsl],
                                    start=st and (kc == 0),
                                    stop=(kc == KC - 1))
                            nc.scalar.copy(out=dstT[:, mc, tsl], in_=pq[:])
                        units.append(qkv_unit)
                for tch in range(NT):
                    def v_unit(tch=tch):
                        t128 = slice(tch * 128, (tch + 1) * 128)
                        pv = ps.tile([128, 128], F32, tag="A", bufs=6,
                                     name="pv")
                        st = True
                        if with_bias:
                            nc.tensor.matmul(pv, onesb_s[0:1, 0:128],
                                             biases["bv"][0:1, msl],
                                             start=True, stop=False)
                            st = False
                        for kc in range(KC):
                            nc.tensor.matmul(
                                pv, ALN[:, kc, t128], wv_t[:, kc, msl],
                                start=st and (kc == 0), stop=(kc == KC - 1))
                        dstv = Vt[:, tch, mc * HPC * VS: (mc + 1) * HPC * VS]
                        dstv = dstv.rearrange("p (h c) -> p h c",
                                              c=VS)[:, :, 0:HD]
                        nc.scalar.copy(
                            out=dstv,
                            in_=pv.rearrange("p (h c) -> p h c", c=HD))
                    units.append(v_unit)
                return units

            def make_proj_units(tci):
                """attn out-projection + bf16 residual for one token chunk."""
                tsl = slice(tci * TS, (tci + 1) * TS)
                units = []
                for mc in range(MC):
                    wt = wpool.tile([128, KC, 128], BF16, tag="w_p", bufs=3,
                                    name="wt_p")
                    nc.sync.dma_start(out=wt, in_=wp_d[mc])

                    def proj_unit(mc=mc, wt=wt):
                        msl = slice(mc * 128, (mc + 1) * 128)
                        po = ps.tile([128, TS], F32, tag="A", bufs=6,
                                     name="po")
                        st = bias_mm(po, biases.get("bp"), msl, tsl)
                        for kc in range(KC):
                            nc.tensor.matmul(
                                po, wt[:, kc, :], YT[:, kc, tsl],
                                start=st and (kc == 0), stop=(kc == KC - 1))
                        nc.vector.tensor_tensor(
                            X1bf[:, mc, tsl], X[:, mc, tsl], po[:], OP.add)
                    units.append(proj_unit)
                return units

            # ---- attention ----
            # Per pair mc, per query chunk qc: PV runs two key-blocks behind
            # the scores so the scalar exp latency hides under score+filler
            # matmuls.  One filler unit (next pair's QKV/V, or the last pair's
            # out-projection) is emitted per key-block iteration.
            def softmax_norm(pys, mc, qc):
                """YT[:, mc, qsl] = pys[half][:HD] / pys[half][HD]; the
                reciprocal denominator rows are PE-broadcast (K=1 matmuls
                into the two partition halves of one psum bank)."""
                qsl = slice(qc * TQ, (qc + 1) * TQ)
                bcD = ps.tile([128, TQ], F32, tag="A", bufs=6, name="bcD")
                for half in range(HPC):
                    den = sp.tile([1, TQ], F32, tag="frow", bufs=3,
                                  name="den")
                    nc.scalar.copy(out=den, in_=pys[half][HD:HD + 1, :])
                    rr = sp.tile([1, TQ], F32, tag="frow", bufs=3, name="rr")
                    nc.vector.reciprocal_approx_fast(out=rr, in_=den)
                    rrb = sp.tile([1, TQ], BF16, tag="brow", bufs=3,
                                  name="rrb")
                    nc.vector.tensor_copy(out=rrb, in_=rr)
                    hsl = slice(half * HD, (half + 1) * HD)
                    nc.tensor.matmul(bcD[hsl, :], onesrow_s[0:1, 0:HD], rrb,
                                     start=True, stop=True)
                bsb = wkp.tile([128, TQ], BF16, tag="bsb", bufs=2, name="bsb")
                nc.scalar.copy(out=bsb, in_=bcD)
                if debug:
                    nc.sync.dma_start(out=dbg["dBS"][:, mc * 2 + qc, :],
                                      in_=bsb)
                for half in range(HPC):
                    hsl = slice(half * HD, (half + 1) * HD)
                    nc.vector.tensor_tensor(
                        YT[hsl, mc, qsl], pys[half][:HD, :], bsb[hsl, :],
                        OP.mult)

            def attn_qc(mc, qc, filler, fi):
                qoff = qc * TQ
                nblk = (qc + 1) * NTQ
                pys = [ps.tile([VS, TQ], F32, tag="B", bufs=2,
                               name=f"py{qc}_{half}") for half in range(HPC)]
                pending = []

                def emit_pv(tch, rq, pexps):
                    for half in range(HPC):
                        h = mc * HPC + half
                        nc.tensor.matmul(
                            pys[half][:, rq],
                            Vt[:, tch, h * VS: (h + 1) * VS],
                            pexps[half][:, rq],
                            start=(tch == 0), stop=(tch == nblk - 1))

                for tch in range(nblk):
                    dq = max(0, tch - qc * NTQ) * 128
                    rq = slice(dq, TQ)
                    qslr = slice(qoff + dq, qoff + TQ)
                    t128 = slice(tch * 128, (tch + 1) * 128)
                    diag = tch >= qc * NTQ
                    pexps = []
                    for half in range(HPC):
                        hsl = slice(half * HD, (half + 1) * HD)
                        psc = ps.tile([128, TQ], F32, tag="A", bufs=6,
                                      name=f"psc{half}")
                        nc.tensor.matmul(
                            psc[:, rq], KT[hsl, mc, t128],
                            QT[hsl, mc, qslr], start=True, stop=True)
                        pexp = wkp.tile([128, TQ], BF16, tag="pexp",
                                        bufs=6, name="pexp")
                        nc.scalar.activation(out=pexp[:, rq], in_=psc[:, rq],
                                             func=AF.Exp)
                        if diag:
                            nc.vector.tensor_tensor(
                                pexp[:, dq:dq + 128],
                                pexp[:, dq:dq + 128], masks_s[:], OP.mult)
                        pexps.append(pexp)
                    if fi[0] < len(filler):
                        filler[fi[0]](); fi[0] += 1
                    pending.append((tch, rq, pexps))
                    if len(pending) > 2:
                        emit_pv(*pending.pop(0))
                while pending:
                    emit_pv(*pending.pop(0))
                softmax_norm(pys, mc, qc)

            for u in make_filler(0):   # prologue: first pair's QKV/V
                u()
            for mc in range(NPAIR):
                last = mc + 1 >= NPAIR
                fi = [0]
                if not last:
                    filler = make_filler(mc + 1)
                    attn_qc(mc, 0, filler, fi)
                    attn_qc(mc, 1, filler, fi)
                else:
                    attn_qc(mc, 0, [], [0])
                    attn_qc(mc, 1, make_proj_units(0), fi)

            # ---- second proj chunk + LN2 + MLP ----
            st2_0 = ln_stats(X1bf, 0)
            proj1 = make_proj_units(1)
            proj1[0]()
            ln_apply(st2_0, X1bf, A2, 0)   # DVE chain hides under proj mms
            for u in proj1[1:]:
                u()
            st2_1 = ln_stats(X1bf, 1)

            # ---- MLP: fc+gelu feeding cp accumulators, per 512-token half --
            # PSUM: 6 held cp accumulators (tag A) + ph cycling (tag B).
            # The next fc's ph matmuls are emitted between a ph group and its
            # cp group so the gelu latency hides under PE work.
            def mlp_qc(qc, after_fc0=None):
                tsl = slice(qc * TS, (qc + 1) * TS)
                pcs = []          # allocated lazily at the first cp group so
                hgels = [None] * FC   # after_fc0 work can use tag-A psum
                wtcs = [None] * FC

                def emit_cp(fc):
                    if not pcs:
                        for mc in range(MC):
                            pc = ps.tile([128, TS], F32, tag="A", bufs=6,
                                         name=f"pc{mc}")
                            st = bias_mm(pc, biases.get("bc"),
                                         slice(mc * 128, (mc + 1) * 128), tsl)
                            pcs.append((pc, st))
                    final = fc == FC - 1
                    for mc in range(MC):
                        pc, st = pcs[mc]
                        nc.tensor.matmul(
                            pc, wtcs[fc][:, mc, :], hgels[fc],
                            start=st and (fc == 0), stop=final)
                        if final:
                            # drain each output as its accumulator completes
                            ot = wkp.tile([128, TS], F32, tag="ot", bufs=3,
                                          name="ot")
                            nc.vector.tensor_tensor(ot, X1bf[:, mc, tsl],
                                                    pc[:], OP.add)
                            nc.sync.dma_start(out=outT_t[:, mc, tsl], in_=ot)

                for fc in range(FC):
                    fsl = slice(fc * 128, (fc + 1) * 128)
                    wt = wpool.tile([128, KC, 128], BF16, tag="w_f", bufs=3,
                                    name="wt_f")
                    nc.sync.dma_start(out=wt, in_=wf_d[fc])
                    wtc = wpool.tile([128, MC, 128], BF16, tag="w_c", bufs=3,
                                     name="wt_c")
                    nc.sync.dma_start(out=wtc, in_=wc_d[fc])
                    wtcs[fc] = wtc
                    ph = ps.tile([128, TS], F32, tag="B", bufs=2, name="ph")
                    st = bias_mm(ph, biases.get("bf"), fsl, tsl)
                    for kc in range(KC):
                        nc.tensor.matmul(
                            ph, wt[:, kc, :], A2[:, kc, tsl],
                            start=st and (kc == 0), stop=(kc == KC - 1))
                    if fc == 1 and after_fc0 is not None:
                        after_fc0()
                    if fc > 0:
                        emit_cp(fc - 1)
                    hgel = wkp.tile([128, TS], BF16, tag="hgel", bufs=3,
                                    name="hgel")
                    nc.scalar.activation(out=hgel, in_=ph, func=gelu_func)
                    hgels[fc] = hgel
                emit_cp(FC - 1)

            mlp_qc(0, after_fc0=lambda: ln_apply(st2_1, X1bf, A2, 1))
            mlp_qc(1)

            if debug:
                for nm, t in (("dALN", ALN), ("dQT", QT), ("dKT", KT),
                              ("dVt", Vt), ("dYT", YT), ("dX1", X1bf),
                              ("dA2", A2)):
                    nc.sync.dma_start(out=dbg[nm][:], in_=t)

    nc.finalize()
    return nc


# --------------------------------------------------------------------------
# Host-side input prep
# --------------------------------------------------------------------------
def _pack_lhsT(w):
    """[Dk, N] -> [N//128, 128, Dk//128, 128] contiguous lhsT tiles."""
    Dk, N = w.shape
    return np.ascontiguousarray(
        w.reshape(Dk // 128, 128, N // 128, 128).transpose(2, 1, 0, 3))


def prepare_weights(wq, bq, wk, bk, wv, bv, w_proj, b_proj, g1, be1, g2, be2,
                    w_fc, b_fc, w_cp, b_cp):
    """Fold LN affines + qk scale + reshape heads; return packed bf16."""
    bf = ml_dtypes.bfloat16
    H_, D_, HD_ = wq.shape
    scale = 1.0 / np.sqrt(HD_)
    # [H, D, HD] -> [D, H*HD]
    wq2 = wq.transpose(1, 0, 2).reshape(D_, H_ * HD_).astype(np.float64)
    wk2 = wk.transpose(1, 0, 2).reshape(D_, H_ * HD_).astype(np.float64)
    wv2 = wv.transpose(1, 0, 2).reshape(D_, H_ * HD_).astype(np.float64)
    g1 = g1.astype(np.float64); be1 = be1.astype(np.float64)
    g2 = g2.astype(np.float64); be2 = be2.astype(np.float64)
    w_fc64 = w_fc.astype(np.float64)
    # fold LN affine: LN_aff(x) = n(x)*g + be  =>  W' = g[:,None]*W,
    # b' = b + be @ W;  the attention scale multiplies wq/bq.
    arrs = {
        "wq": _pack_lhsT((g1[:, None] * wq2 * scale).astype(bf)),
        "wk": _pack_lhsT((g1[:, None] * wk2).astype(bf)),
        "wv": np.ascontiguousarray(
            (g1[:, None] * wv2).astype(bf)
            .reshape(-1, 128, wv2.shape[1]).transpose(1, 0, 2)),
        "wp": _pack_lhsT(w_proj.astype(bf)),
        "wf": _pack_lhsT((g2[:, None] * w_fc64).astype(bf)),
        "wc": np.ascontiguousarray(
            w_cp.astype(bf).reshape(-1, 128, w_cp.shape[1] // 128, 128)),
    }
    bias_arrs = {
        "bq": (bq.reshape(-1).astype(np.float64) + be1 @ wq2) * scale,
        "bk": bk.reshape(-1).astype(np.float64) + be1 @ wk2,
        "bv": bv.reshape(-1).astype(np.float64) + be1 @ wv2,
        "bp": b_proj.astype(np.float64),
        "bf": b_fc.astype(np.float64) + be2 @ w_fc64,
        "bc": b_cp.astype(np.float64),
    }
    any_bias = bool(any(np.any(v != 0) for v in bias_arrs.values()))
    if any_bias:
        for k, v in bias_arrs.items():
            arrs[k] = v.astype(bf).reshape(1, -1)
    return arrs, any_bias


_NC_CACHE = {}


def kernel(**inputs):
    x = np.asarray(inputs["x"], np.float32)
    arrs, any_bias = prepare_weights(
        *(np.asarray(inputs[k]) for k in (
            "wq", "bq", "wk", "bk", "wv", "bv", "w_proj", "b_proj",
            "g1", "be1", "g2", "be2", "w_fc", "b_fc", "w_cp", "b_cp")))
    key = ("full", any_bias)
    if key not in _NC_CACHE:
        _NC_CACHE[key] = build_decoder_nc(with_bias=any_bias)
    nc = _NC_CACHE[key]

    in_maps = []
    for b in range(N_CORES):
        m = dict(arrs)
        m["xT"] = np.ascontiguousarray(x[b].T)
        in_maps.append(m)

    from concourse.bass_utils import run_bass_kernel_spmd
    res = run_bass_kernel_spmd(nc, in_maps, list(range(N_CORES)))
    out = np.stack([res.results[i]["outT"].T for i in range(N_CORES)])
    return out.astype(np.float32)


# revision 35
# speedup vs baseline: 1.2332x; 1.2332x over previous
"""Trainium2 Bass kernel for an nn.DecoderBlock (pre-LN GPT block).

Reference computation (per batch element, fp32):
    h  = LN(x; g1,be1);  q,k,v = per-head projections of h
    y  = causal-softmax(q k^T / sqrt(hd)) v ;  x1 = x + y @ w_proj + b_proj
    h2 = LN(x1; g2,be2); out = x1 + gelu_tanh(h2 @ w_fc + b_fc) @ w_cp + b_cp

Shapes: B=8, T=1024, D=768, H=12, HD=64, F=3072.

Strategy: pure data parallelism — batch element b runs on core b (B == n_cores
== 8); the decoder block is independent per batch element so no collectives are
needed.  On-chip, all activations are kept *feature-major* ([D, T]: features on
partitions, tokens on the free axis) so chained matmuls need no transposes:
    out^T[n, t] = sum_d W[d, n] * A^T[d, t]   (lhsT = W as stored, rhs = A^T)
Attention scores are computed transposed (S^T[t, q]) so the softmax-weighted
probabilities land directly in the [t, q] layout the P@V matmul needs as its
moving operand; the two heads sharing a 128-partition group issue their K=64
score matmuls back-to-back so the PE runs them concurrently in disjoint
row-groups.  The softmax denominator comes from augmenting V with a
ones-column (row HD of the PV output is sum_t P[t,q]).  Softmax max-subtraction
is skipped: post-LN scores are O(5) so fp32 exp cannot overflow.

Perf notes (vs the first working version):
  * No GpSimd anywhere: partition broadcasts are K=1/K=2 PE matmuls against
    ones/pattern vectors; the causal-mask multiply runs on DVE.  GpSimd ops
    carried ~1us semaphore overhead each and serialized whole attention pairs.
  * PV matmuls run two iterations behind their score matmuls so the scalar
    engine's exp latency is hidden by score+filler matmuls.
  * LayerNorm stats stay in row layout ([1, T] psum rows from +-1/D ones
    matmuls); the per-token factors are broadcast across partitions by the PE.
  * MLP: the next fc tile's matmuls are emitted between a ph group and its
    dependent cp group, hiding the gelu; PSUM = 6 held cp accumulators + 2
    cycling scratch banks.
  * 1/sqrt(hd) folded into wq host-side; residual carried in bf16 (X1bf);
    ACT tables (Square/Sqrt/Exp/Gelu) prewarmed during the x DMA.
"""

import numpy as np
import ml_dtypes

import concourse.bass as bass
import concourse.mybir as mybir
import concourse.tile as tile
from concourse import bacc

BF16 = mybir.dt.bfloat16
F32 = mybir.dt.float32
AF = mybir.ActivationFunctionType
OP = mybir.AluOpType

# Full-problem dimensions (hardcoded; harness contract).
B, T, D, H = 8, 1024, 768, 12
HD = D // H
F = 4 * D
EPS = 1e-5
N_CORES = 8


# --------------------------------------------------------------------------
# Bass program builder
# --------------------------------------------------------------------------
def build_decoder_nc(T=T, D=D, H=H, F=F, TQ=512, with_bias=False, eps=EPS,
                     gelu_func=AF.Gelu_apprx_tanh, debug=False):
    """Build the single-core Bass program (same program runs SPMD on all cores).

    DRAM I/O layouts (all prepared host-side):
      xT    [D, T]             f32   x^T (feature-major)
      wq,wk [MC,128,KC,128]    bf16  packed lhsT tiles (LN1 affine + qk scale
                                     folded in)
      wv    [128,KC,D]         bf16  rhs layout for token-major V
      wp    [MC,128,KC,128]    bf16  w_proj packed
      wf    [FC,128,KC,128]    bf16  w_fc packed (LN2 affine folded in)
      wc    [FC,128,MC,128]    bf16  w_cp packed fc-major (plain reshape)
      *_b   [1, N]             bf16  folded bias rows (only if with_bias)
      outT  [D, T]             f32   output^T
    """
    assert D % 128 == 0 and F % 128 == 0 and T % TQ == 0 and TQ % 128 == 0
    TS = min(512, T)           # token chunk for projections/LN stats
    assert T % TS == 0 and TS == TQ
    KC = D // 128          # contraction chunks over D
    FC = F // 128          # chunks over MLP hidden
    MC = D // 128          # output-feature chunks over D
    NT = T // 128          # key/token chunks of 128
    NQ = T // TQ           # query chunks of TQ
    NTQ = TQ // 128
    HPC = 128 // HD        # heads per 128-partition group (2 for HD=64)
    VS = HD + 1            # V columns per head incl. ones-column
    assert H % HPC == 0 and HPC == 2 and NQ == 2
    NPAIR = H // HPC
    assert NPAIR == MC

    nc = bacc.Bacc()

    # ---- DRAM I/O ----
    xT = nc.dram_tensor("xT", [D, T], F32, kind="ExternalInput")
    wq_d = nc.dram_tensor("wq", [MC, 128, KC, 128], BF16, kind="ExternalInput")
    wk_d = nc.dram_tensor("wk", [MC, 128, KC, 128], BF16, kind="ExternalInput")
    wv_d = nc.dram_tensor("wv", [128, KC, D], BF16, kind="ExternalInput")
    wp_d = nc.dram_tensor("wp", [MC, 128, KC, 128], BF16, kind="ExternalInput")
    wf_d = nc.dram_tensor("wf", [FC, 128, KC, 128], BF16, kind="ExternalInput")
    wc_d = nc.dram_tensor("wc", [FC, 128, MC, 128], BF16, kind="ExternalInput")
    bias_d = {}
    if with_bias:
        for nm, width in (("bq", D), ("bk", D), ("bv", D), ("bp", D),
                          ("bf", F), ("bc", D)):
            bias_d[nm] = nc.dram_tensor(nm, [1, width], BF16,
                                        kind="ExternalInput")
    outT = nc.dram_tensor("outT", [D, T], F32, kind="ExternalOutput")
    outT_t = outT[:].rearrange("(o p) t -> p o t", p=128)
    dbg = {}
    if debug:
        NT_ = T // 128
        VS_ = HD + 1
        for nm, shp in (("dALN", [128, D // 128, T]), ("dQT", [128, D // 128, T]),
                        ("dKT", [128, D // 128, T]), ("dVt", [128, NT_, H * VS_]),
                        ("dYT", [128, D // 128, T]), ("dX1", [128, D // 128, T]),
                        ("dA2", [128, D // 128, T])):
            dbg[nm] = nc.dram_tensor(nm, shp, BF16, kind="ExternalOutput")
        dbg["dBS"] = nc.dram_tensor("dBS", [128, (H // 2) * 2, TQ], BF16,
                                    kind="ExternalOutput")

    # ---- constants (embedded in the NEFF) ----
    ones_bf = nc.inline_tensor(np.ones((1, T), ml_dtypes.bfloat16), "ones_bf")
    # stats ones-columns, mean factors folded: row0 = -1/D (mu), row1 = +1/D
    oc_np = np.empty((128, 2), ml_dtypes.bfloat16)
    oc_np[:, 0] = -1.0 / D
    oc_np[:, 1] = 1.0 / D
    onescol = nc.inline_tensor(oc_np, "onescol")
    # triangular mask for the diagonal 128x128 score blocks: 1 if i <= j
    m_np = (np.arange(128)[:, None] <= np.arange(128)[None, :]).astype(
        ml_dtypes.bfloat16)
    masks_d = nc.inline_tensor(m_np, "masks")

    with tile.TileContext(nc) as tc:
        with (
            tc.tile_pool(name="persist", bufs=1) as pp,
            tc.tile_pool(name="wts", bufs=3) as wpool,
            tc.tile_pool(name="work", bufs=3) as wkp,
            tc.tile_pool(name="small", bufs=2) as sp,
            tc.tile_pool(name="ps", bufs=1, space="PSUM") as ps,
        ):
            # ---- persistent SBUF tensors ----
            X = pp.tile([128, KC, T], F32, tag="X", name="X")
            Xbf = pp.tile([128, KC, T], BF16, tag="Xbf", name="Xbf")
            ALN = pp.tile([128, KC, T], BF16, tag="ALN", name="ALN")
            QT = pp.tile([128, KC, T], BF16, tag="QT", name="QT")
            KT = pp.tile([128, KC, T], BF16, tag="KT", name="KT")
            Vt = pp.tile([128, NT, H * VS], BF16, tag="Vt", name="Vt")
            YT = pp.tile([128, KC, T], BF16, tag="YT", name="YT")
            X1bf = pp.tile([128, KC, T], BF16, tag="X1bf", name="X1bf")
            A2 = pp.tile([128, KC, T], BF16, tag="A2", name="A2")

            onesb_s = None
            if with_bias:
                onesb_s = pp.tile([1, T], BF16, tag="onesb", name="onesb_s")
                nc.sync.dma_start(out=onesb_s, in_=ones_bf[:])
            onescol_s = pp.tile([128, 2], BF16, tag="onescol",
                                name="onescol_s")
            nc.sync.dma_start(out=onescol_s, in_=onescol[:])
            onesrow_s = pp.tile([1, 128], BF16, tag="onesrow",
                                name="onesrow_s")
            nc.sync.dma_start(out=onesrow_s, in_=ones_bf[0:1, 0:128])
            eps_p = pp.tile([1, 1], F32, tag="eps", name="eps_p")
            nc.vector.memset(eps_p, eps)
            masks_s = pp.tile([128, 128], BF16, tag="masks", name="masks_s")
            nc.sync.dma_start(out=masks_s, in_=masks_d[:])
            biases = {}
            for nm, dten in bias_d.items():
                bt = pp.tile(list(dten.shape), BF16, tag=nm, name=f"{nm}_s")
                nc.sync.dma_start(out=bt, in_=dten[:])
                biases[nm] = bt

            # ---- ACT table prewarm (overlaps the x DMA) ----
            warm = pp.tile([1, 5], F32, tag="warm", name="warm")
            nc.vector.memset(warm, 1.0)

            def prewarm(i, fn):
                nc.scalar.activation(out=warm[0:1, i:i + 1],
                                     in_=warm[0:1, i:i + 1], func=fn)

            prewarm(0, AF.Square)
            prewarm(1, AF.Sqrt)
            prewarm(2, AF.Copy)

            # V weights resident; Vt ones-columns
            wv_t = pp.tile([128, KC, D], BF16, tag="wv", name="wv_t")
            nc.sync.dma_start(out=wv_t, in_=wv_d[:])
            for h in range(H):
                nc.vector.memset(Vt[:, :, h * VS + HD: h * VS + HD + 1], 1.0)

            # ---- load x^T (per-kc pipelining: cast as each chunk lands) ----
            xT_t = xT[:].rearrange("(o p) t -> p o t", p=128)
            for kc in range(KC):
                nc.sync.dma_start(out=X[:, kc, :], in_=xT_t[:, kc, :])
                nc.vector.tensor_copy(out=Xbf[:, kc, :], in_=X[:, kc, :])

            # ---- LayerNorm (row layout; PE broadcasts; no GpSimd) ----
            # st[0,:] = -mu, st[32,:] = E[x^2] via +-1/D ones matmuls
            # (engine APs must start at partition 0/32/64).
            def ln_stats_kc(srcbf, tci, st, kc):
                tsl = slice(tci * TS, (tci + 1) * TS)
                sqc = wkp.tile([128, TS], BF16, tag="sqc", bufs=3,
                               name="sqc")
                nc.scalar.activation(out=sqc, in_=srcbf[:, kc, tsl],
                                     func=AF.Square)
                nc.tensor.matmul(
                    st[0:1, :], onescol_s[:, 0:1], srcbf[:, kc, tsl],
                    start=(kc == 0), stop=(kc == KC - 1))
                nc.tensor.matmul(
                    st[32:33, :], onescol_s[:, 1:2], sqc,
                    start=(kc == 0), stop=(kc == KC - 1))

            def ln_stats(srcbf, tci):
                st = ps.tile([33, TS], F32, tag="B", bufs=2, name="st")
                for kc in range(KC):
                    ln_stats_kc(srcbf, tci, st, kc)
                return st

            def ln_apply(st, srcbf, dst, tci):
                """Row chain: rstd/-mu*rstd rows -> PE broadcast -> DVE."""
                tsl = slice(tci * TS, (tci + 1) * TS)
                musq = sp.tile([1, TS], F32, tag="frow", bufs=3, name="musq")
                nc.scalar.activation(out=musq, in_=st[0:1, :],
                                     func=AF.Square)
                sdev = sp.tile([1, TS], F32, tag="frow", bufs=3, name="sdev")
                nc.vector.tensor_tensor(sdev, st[32:33, :], musq, OP.subtract)
                nc.scalar.activation(out=sdev, in_=sdev,
                                     func=AF.Sqrt, bias=eps_p[:])
                rstd = sp.tile([1, TS], F32, tag="frow", bufs=3, name="rstd")
                nc.vector.reciprocal_approx_fast(out=rstd, in_=sdev)
                rbf = sp.tile([1, TS], BF16, tag="brow", bufs=3, name="rbf")
                nc.vector.tensor_copy(out=rbf, in_=rstd)
                nbf = sp.tile([1, TS], BF16, tag="brow", bufs=3, name="nbf")
                # nbf = (-mu) * rstd
                nc.vector.tensor_tensor(nbf, st[0:1, :], rstd, OP.mult)
                bcR = ps.tile([128, TS], F32, tag="A", bufs=6, name="bcR")
                bcN = ps.tile([128, TS], F32, tag="A", bufs=6, name="bcN")
                nc.tensor.matmul(bcR, onesrow_s[0:1, :], rbf,
                                 start=True, stop=True)
                nc.tensor.matmul(bcN, onesrow_s[0:1, :], nbf,
                                 start=True, stop=True)
                # psum->sbuf on the scalar engine (closer to PSUM; keeps the
                # big DVE multiplies in packed bf16-SBUF mode)
                bcRs = wkp.tile([128, TS], BF16, tag="bcRs", bufs=2,
                                name="bcRs")
                bcNs = wkp.tile([128, TS], BF16, tag="bcNs", bufs=2,
                                name="bcNs")
                nc.scalar.copy(out=bcRs, in_=bcR)
                nc.scalar.copy(out=bcNs, in_=bcN)
                tmp = wkp.tile([128, KC, TS], BF16, tag="lntmp", bufs=2,
                               name="lntmp")
                nc.vector.tensor_tensor(
                    tmp, srcbf[:, :, tsl],
                    bcRs[:, None, :].to_broadcast((128, KC, TS)), OP.mult)
                nc.vector.tensor_tensor(
                    dst[:, :, tsl], tmp,
                    bcNs[:, None, :].to_broadcast((128, KC, TS)), OP.add)

            # LN1: both chunks' stats stream per-kc with the x DMA, then the
            # row chains; Exp/Gelu tables load during the stats matmuls.
            st0 = ps.tile([33, TS], F32, tag="B", bufs=2, name="st0")
            st1 = ps.tile([33, TS], F32, tag="B", bufs=2, name="st1")
            for kc in range(KC):
                ln_stats_kc(Xbf, 0, st0, kc)
                ln_stats_kc(Xbf, 1, st1, kc)
            ln_apply(st0, Xbf, ALN, 0)
            ln_apply(st1, Xbf, ALN, 1)
            prewarm(3, AF.Exp)
            prewarm(4, gelu_func)

            # ---- QKV / V / proj unit builders ----
            def bias_mm(psum, bias_t, msl, tsl):
                if bias_t is None:
                    return True
                nc.tensor.matmul(psum, bias_t[0:1, msl], onesb_s[0:1, tsl],
                                 start=True, stop=False)
                return False

            def make_filler(mc):
                """Filler units (closures) for pair mc's QKV + V matmuls."""
                msl = slice(mc * 128, (mc + 1) * 128)
                units = []
                for nm, wten, dstT in (("bq", wq_d, QT), ("bk", wk_d, KT)):
                    wt = wpool.tile([128, KC, 128], BF16, tag="w_qk", bufs=3,
                                    name="wt_qk")
                    nc.sync.dma_start(out=wt, in_=wten[mc])
                    for tci in range(NQ):
                        def qkv_unit(nm=nm, wt=wt, dstT=dstT, tci=tci):
                            tsl = slice(tci * TS, (tci + 1) * TS)
                            pq = ps.tile([128, TS], F32, tag="A", bufs=6,
                                         name="pq")
                            st = bias_mm(pq, biases.get(nm), msl, tsl)
                            for kc in range(KC):
                                nc.tensor.matmul(
                                    pq, wt[:, kc, :], ALN[:, kc, tsl],
                                    start=st and (kc == 0),
                                    stop=(kc == KC - 1))
                            nc.vector.tensor_copy(out=dstT[:, mc, tsl],
                                                  in_=pq[:])
                        units.append(qkv_unit)
                for tch in range(NT):
                    def v_unit(tch=tch):
                        t128 = slice(tch * 128, (tch + 1) * 128)
                        pv = ps.tile([128, 128], F32, tag="A", bufs=6,
                                     name="pv")
                        st = True
                        if with_bias:
                            nc.tensor.matmul(pv, onesb_s[0:1, 0:128],
                                             biases["bv"][0:1, msl],
                                             start=True, stop=False)
                            st = False
                        for kc in range(KC):
                            nc.tensor.matmul(
                                pv, ALN[:, kc, t128], wv_t[:, kc, msl],
                                start=st and (kc == 0), stop=(kc == KC - 1))
                        dstv = Vt[:, tch, mc * HPC * VS: (mc + 1) * HPC * VS]
                        dstv = dstv.rearrange("p (h c) -> p h c",
                                              c=VS)[:, :, 0:HD]
                        nc.vector.tensor_copy(
                            out=dstv,
                            in_=pv.rearrange("p (h c) -> p h c", c=HD))
                    units.append(v_unit)
                return units

            def make_proj_units(tci):
                """attn out-projection + bf16 residual for one token chunk."""
                tsl = slice(tci * TS, (tci + 1) * TS)
                units = []
                for mc in range(MC):
                    wt = wpool.tile([128, KC, 128], BF16, tag="w_p", bufs=3,
                                    name="wt_p")
                    nc.sync.dma_start(out=wt, in_=wp_d[mc])

                    def proj_unit(mc=mc, wt=wt):
                        msl = slice(mc * 128, (mc + 1) * 128)
                        po = ps.tile([128, TS], F32, tag="A", bufs=6,
                                     name="po")
                        st = bias_mm(po, biases.get("bp"), msl, tsl)
                        for kc in range(KC):
                            nc.tensor.matmul(
                                po, wt[:, kc, :], YT[:, kc, tsl],
                                start=st and (kc == 0), stop=(kc == KC - 1))
                        nc.vector.tensor_tensor(
                            X1bf[:, mc, tsl], X[:, mc, tsl], po[:], OP.add)
                    units.append(proj_unit)
                return units

            # ---- attention ----
            # Per pair mc, per query chunk qc: PV runs two key-blocks behind
            # the scores so the scalar exp latency hides under score+filler
            # matmuls.  One filler unit (next pair's QKV/V, or the last pair's
            # out-projection) is emitted per key-block iteration.
            def softmax_norm(pys, mc, qc):
                """YT[:, mc, qsl] = pys[half][:HD] / pys[half][HD]; the
                reciprocal denominator rows are PE-broadcast (K=1 matmuls
                into the two partition halves of one psum bank)."""
                qsl = slice(qc * TQ, (qc + 1) * TQ)
                bcD = ps.tile([128, TQ], F32, tag="A", bufs=6, name="bcD")
                for half in range(HPC):
                    den = sp.tile([1, TQ], F32, tag="frow", bufs=3,
                                  name="den")
                    nc.scalar.copy(out=den, in_=pys[half][HD:HD + 1, :])
                    rr = sp.tile([1, TQ], F32, tag="frow", bufs=3, name="rr")
                    nc.vector.reciprocal_approx_fast(out=rr, in_=den)
                    rrb = sp.tile([1, TQ], BF16, tag="brow", bufs=3,
                                  name="rrb")
                    nc.vector.tensor_copy(out=rrb, in_=rr)
                    hsl = slice(half * HD, (half + 1) * HD)
                    nc.tensor.matmul(bcD[hsl, :], onesrow_s[0:1, 0:HD], rrb,
                                     start=True, stop=True)
                bsb = wkp.tile([128, TQ], BF16, tag="bsb", bufs=2, name="bsb")
                nc.scalar.copy(out=bsb, in_=bcD)
                if debug:
                    nc.sync.dma_start(out=dbg["dBS"][:, mc * 2 + qc, :],
                                      in_=bsb)
                for half in range(HPC):
                    hsl = slice(half * HD, (half + 1) * HD)
                    nc.vector.tensor_tensor(
                        YT[hsl, mc, qsl], pys[half][:HD, :], bsb[hsl, :],
                        OP.mult)

            def attn_qc(mc, qc, filler, fi):
                qoff = qc * TQ
                nblk = (qc + 1) * NTQ
                pys = [ps.tile([VS, TQ], F32, tag="B", bufs=2,
                               name=f"py{qc}_{half}") for half in range(HPC)]
                pending = []

                def emit_pv(tch, rq, pexps):
                    for half in range(HPC):
                        h = mc * HPC + half
                        nc.tensor.matmul(
                            pys[half][:, rq],
                            Vt[:, tch, h * VS: (h + 1) * VS],
                            pexps[half][:, rq],
                            start=(tch == 0), stop=(tch == nblk - 1))

                for tch in range(nblk):
                    dq = max(0, tch - qc * NTQ) * 128
                    rq = slice(dq, TQ)
                    qslr = slice(qoff + dq, qoff + TQ)
                    t128 = slice(tch * 128, (tch + 1) * 128)
                    diag = tch >= qc * NTQ
                    pexps = []
                    for half in range(HPC):
                        hsl = slice(half * HD, (half + 1) * HD)
                        psc = ps.tile([128, TQ], F32, tag="A", bufs=6,
                                      name=f"psc{half}")
                        nc.tensor.matmul(
                            psc[:, rq], KT[hsl, mc, t128],
                            QT[hsl, mc, qslr], start=True, stop=True)
                        pexp = wkp.tile([128, TQ], BF16, tag="pexp",
                                        bufs=6, name="pexp")
                        nc.scalar.activation(out=pexp[:, rq], in_=psc[:, rq],
                                             func=AF.Exp)
                        if diag:
                            nc.vector.tensor_tensor(
                                pexp[:, dq:dq + 128],
                                pexp[:, dq:dq + 128], masks_s[:], OP.mult)
                        pexps.append(pexp)
                    if fi[0] < len(filler):
                        filler[fi[0]](); fi[0] += 1
                    pending.append((tch, rq, pexps))
                    if len(pending) > 2:
                        emit_pv(*pending.pop(0))
                while pending:
                    emit_pv(*pending.pop(0))
                softmax_norm(pys, mc, qc)

            for u in make_filler(0):   # prologue: first pair's QKV/V
                u()
            for mc in range(NPAIR):
                last = mc + 1 >= NPAIR
                fi = [0]
                if not last:
                    filler = make_filler(mc + 1)
                    attn_qc(mc, 0, filler, fi)
                    attn_qc(mc, 1, filler, fi)
                else:
                    attn_qc(mc, 0, [], [0])
                    attn_qc(mc, 1, make_proj_units(0), fi)

            # ---- second proj chunk + LN2 + MLP ----
            st2_0 = ln_stats(X1bf, 0)
            proj1 = make_proj_units(1)
            proj1[0]()
            ln_apply(st2_0, X1bf, A2, 0)   # DVE chain hides under proj mms
            for u in proj1[1:]:
                u()
            st2_1 = ln_stats(X1bf, 1)

            # ---- MLP: fc+gelu feeding cp accumulators, per 512-token half --
            # PSUM: 6 held cp accumulators (tag A) + ph cycling (tag B).
            # The next fc's ph matmuls are emitted between a ph group and its
            # cp group so the gelu latency hides under PE work.
            def mlp_qc(qc, after_fc0=None):
                tsl = slice(qc * TS, (qc + 1) * TS)
                pcs = []          # allocated lazily at the first cp group so
                hgels = [None] * FC   # after_fc0 work can use tag-A psum
                wtcs = [None] * FC

                def emit_cp(fc):
                    if not pcs:
                        for mc in range(MC):
                            pc = ps.tile([128, TS], F32, tag="A", bufs=6,
                                         name=f"pc{mc}")
                            st = bias_mm(pc, biases.get("bc"),
                                         slice(mc * 128, (mc + 1) * 128), tsl)
                            pcs.append((pc, st))
                    final = fc == FC - 1
                    for mc in range(MC):
                        pc, st = pcs[mc]
                        nc.tensor.matmul(
                            pc, wtcs[fc][:, mc, :], hgels[fc],
                            start=st and (fc == 0), stop=final)
                        if final:
                            # drain each output as its accumulator completes
                            ot = wkp.tile([128, TS], F32, tag="ot", bufs=3,
                                          name="ot")
                            nc.vector.tensor_tensor(ot, X1bf[:, mc, tsl],
                                                    pc[:], OP.add)
                            nc.sync.dma_start(out=outT_t[:, mc, tsl], in_=ot)

                for fc in range(FC):
                    fsl = slice(fc * 128, (fc + 1) * 128)
                    wt = wpool.tile([128, KC, 128], BF16, tag="w_f", bufs=3,
                                    name="wt_f")
                    nc.sync.dma_start(out=wt, in_=wf_d[fc])
                    wtc = wpool.tile([128, MC, 128], BF16, tag="w_c", bufs=3,
                                     name="wt_c")
                    nc.sync.dma_start(out=wtc, in_=wc_d[fc])
                    wtcs[fc] = wtc
                    ph = ps.tile([128, TS], F32, tag="B", bufs=2, name="ph")
                    st = bias_mm(ph, biases.get("bf"), fsl, tsl)
                    for kc in range(KC):
                        nc.tensor.matmul(
                            ph, wt[:, kc, :], A2[:, kc, tsl],
                            start=st and (kc == 0), stop=(kc == KC - 1))
                    if fc == 1 and after_fc0 is not None:
                        after_fc0()
                    if fc > 0:
                        emit_cp(fc - 1)
                    hgel = wkp.tile([128, TS], BF16, tag="hgel", bufs=3,
                                    name="hgel")
                    nc.scalar.activation(out=hgel, in_=ph, func=gelu_func)
                    hgels[fc] = hgel
                emit_cp(FC - 1)

            mlp_qc(0, after_fc0=lambda: ln_apply(st2_1, X1bf, A2, 1))
            mlp_qc(1)

            if debug:
                for nm, t in (("dALN", ALN), ("dQT", QT), ("dKT", KT),
                              ("dVt", Vt), ("dYT", YT), ("dX1", X1bf),
                              ("dA2", A2)):
                    nc.sync.dma_start(out=dbg[nm][:], in_=t)

    nc.finalize()
    return nc


# --------------------------------------------------------------------------
# Host-side input prep
# --------------------------------------------------------------------------
def _pack_lhsT(w):
    """[Dk, N] -> [N//128, 128, Dk//128, 128] contiguous lhsT tiles."""
    Dk, N = w.shape
    return np.ascontiguousarray(
        w.reshape(Dk // 128, 128, N // 128, 128).transpose(2, 1, 0, 3))


def prepare_weights(wq, bq, wk, bk, wv, bv, w_proj, b_proj, g1, be1, g2, be2,
                    w_fc, b_fc, w_cp, b_cp):
    """Fold LN affines + qk scale + reshape heads; return packed bf16."""
    bf = ml_dtypes.bfloat16
    H_, D_, HD_ = wq.shape
    scale = 1.0 / np.sqrt(HD_)
    # [H, D, HD] -> [D, H*HD]
    wq2 = wq.transpose(1, 0, 2).reshape(D_, H_ * HD_).astype(np.float64)
    wk2 = wk.transpose(1, 0, 2).reshape(D_, H_ * HD_).astype(np.float64)
    wv2 = wv.transpose(1, 0, 2).reshape(D_, H_ * HD_).astype(np.float64)
    g1 = g1.astype(np.float64); be1 = be1.astype(np.float64)
    g2 = g2.astype(np.float64); be2 = be2.astype(np.float64)
    w_fc64 = w_fc.astype(np.float64)
    # fold LN affine: LN_aff(x) = n(x)*g + be  =>  W' = g[:,None]*W,
    # b' = b + be @ W;  the attention scale multiplies wq/bq.
    arrs = {
        "wq": _pack_lhsT((g1[:, None] * wq2 * scale).astype(bf)),
        "wk": _pack_lhsT((g1[:, None] * wk2).astype(bf)),
        "wv": np.ascontiguousarray(
            (g1[:, None] * wv2).astype(bf)
            .reshape(-1, 128, wv2.shape[1]).transpose(1, 0, 2)),
        "wp": _pack_lhsT(w_proj.astype(bf)),
        "wf": _pack_lhsT((g2[:, None] * w_fc64).astype(bf)),
        "wc": np.ascontiguousarray(
            w_cp.astype(bf).reshape(-1, 128, w_cp.shape[1] // 128, 128)),
    }
    bias_arrs = {
        "bq": (bq.reshape(-1).astype(np.float64) + be1 @ wq2) * scale,
        "bk": bk.reshape(-1).astype(np.float64) + be1 @ wk2,
        "bv": bv.reshape(-1).astype(np.float64) + be1 @ wv2,
        "bp": b_proj.astype(np.float64),
        "bf": b_fc.astype(np.float64) + be2 @ w_fc64,
        "bc": b_cp.astype(np.float64),
    }
    any_bias = bool(any(np.any(v != 0) for v in bias_arrs.values()))
    if any_bias:
        for k, v in bias_arrs.items():
            arrs[k] = v.astype(bf).reshape(1, -1)
    return arrs, any_bias


_NC_CACHE = {}


def kernel(**inputs):
    x = np.asarray(inputs["x"], np.float32)
    arrs, any_bias = prepare_weights(
        *(np.asarray(inputs[k]) for k in (
            "wq", "bq", "wk", "bk", "wv", "bv", "w_proj", "b_proj",
            "g1", "be1", "g2", "be2", "w_fc", "b_fc", "w_cp", "b_cp")))
    key = ("full", any_bias)
    if key not in _NC_CACHE:
        _NC_CACHE[key] = build_decoder_nc(with_bias=any_bias)
    nc = _NC_CACHE[key]

    in_maps = []
    for b in range(N_CORES):
        m = dict(arrs)
        m["xT"] = np.ascontiguousarray(x[b].T)
        in_maps.append(m)

    from concourse.bass_utils import run_bass_kernel_spmd
    res = run_bass_kernel_spmd(nc, in_maps, list(range(N_CORES)))
    out = np.stack([res.results[i]["outT"].T for i in range(N_CORES)])
    return out.astype(np.float32)
